# revision 1
# baseline (speedup 1.0000x reference)
"""Trainium2 Bass kernel for nn_ExpertGroup (moe_routing).

Sharding: 8 cores = (batch b in 0..3) x (seq half j in 0..1); each core owns
1024 tokens. Activations flow in transposed [feature, token] layout so every
matmul contracts over the partition dim. The sequence-mixing adapt attention
needs full-S adapt_in/adapt_out, so paired cores AllGather their N-layout
halves, split per 512-token half so transfers overlap compute.

v2 restructure vs baseline:
- up/gate loop is token-half-outer (n outer, ht inner) with wu/wg fully
  SBUF-resident, so the first half's adapt_out collective fires at ~half
  time and P6 overlaps the transfer of the second.
- Wc=(0.1*Wo@Wp).T and Wcd=(0.1*Wd@Wap).T are folded on the host; the
  adapt@Wap.T hidden update (old P7) collapses into one PSUM-accumulated
  matmul per output tile (wd group), and the expert contrib likewise.
- Expert branch (matmuls + LN stats) interleaves into the second up/gate
  half; LN sqrt ops are batched (2 sites) to minimize Act table reloads,
  and elementwise expert chains run on the Pool engine.
"""

import numpy as np
import ml_dtypes

import concourse.bacc as bacc
import concourse.mybir as mybir
import concourse.tile as tile
from concourse import bass_utils

F32 = mybir.dt.float32
BF16 = mybir.dt.bfloat16
AX = mybir.AxisListType
OP = mybir.AluOpType
AF = mybir.ActivationFunctionType

B, S, D, H, AD, E = 4, 2048, 1024, 2048, 128, 8
PHASES = []


def _mark(nc, name):
    PHASES.append((name, nc.next_id()))

TOK = 1024          # tokens per core
N_CORES = 8
NCH = TOK // 512    # 512-wide matmul chunks of the own token range
BF = ml_dtypes.bfloat16

_NC_CACHE = None


def build(fake_cc=False):
    nc = bacc.Bacc("TRN2", target_bir_lowering=False, debug=False,
                   num_devices=N_CORES)

    # ---- per-core DRAM parameters ----
    xt = nc.declare_dram_parameter("xt", [D, TOK], BF16, isOutput=False)
    ew = nc.declare_dram_parameter("ew", [TOK, E], F32, isOutput=False)
    ewt = nc.declare_dram_parameter("ewt", [E, TOK], F32, isOutput=False)
    wu_t = nc.declare_dram_parameter("wu_t", [16, 128, 8, 128], BF16, isOutput=False)
    wg_t = nc.declare_dram_parameter("wg_t", [16, 128, 8, 128], BF16, isOutput=False)
    wd_t = nc.declare_dram_parameter("wd_t", [8, 128, 16, 128], BF16, isOutput=False)
    wpre_t = nc.declare_dram_parameter("wpre_t", [D, AD], BF16, isOutput=False)
    wpost_t = nc.declare_dram_parameter("wpost_t", [H, AD], BF16, isOutput=False)
    wc = nc.declare_dram_parameter("wc", [AD, D], BF16, isOutput=False)    # (0.1*Wo@Wp).T
    wcd = nc.declare_dram_parameter("wcd", [AD, D], BF16, isOutput=False)  # (0.1*Wd@Wap).T
    a_t = nc.declare_dram_parameter("a_t", [E, AD, AD], BF16, isOutput=False)
    bu = nc.declare_dram_parameter("bu", [H], F32, isOutput=False)
    bg = nc.declare_dram_parameter("bg", [H], F32, isOutput=False)
    bd = nc.declare_dram_parameter("bd", [D], F32, isOutput=False)
    bpre = nc.declare_dram_parameter("bpre", [AD], F32, isOutput=False)
    bpost = nc.declare_dram_parameter("bpost", [AD], F32, isOutput=False)
    ln_g = nc.declare_dram_parameter("ln_g", [AD], F32, isOutput=False)
    ln_b = nc.declare_dram_parameter("ln_b", [AD], F32, isOutput=False)
    eg = nc.declare_dram_parameter("eg", [E, AD], F32, isOutput=False)
    eb = nc.declare_dram_parameter("eb", [E, AD], F32, isOutput=False)
    id_f32 = nc.declare_dram_parameter("id_f32", [128, 128], F32, isOutput=False)
    out = nc.declare_dram_parameter("out", [D, TOK], F32, isOutput=True)

    with tile.TileContext(nc) as tc:
        _emit(nc, tc, locals(), fake_cc)
    nc.compile()
    return nc


def _emit(nc, tc, P, fake_cc=False):
    xt, ew, ewt = P["xt"], P["ew"], P["ewt"]
    wu_t, wg_t, wd_t = P["wu_t"], P["wg_t"], P["wd_t"]
    wpre_t, wpost_t, wc_p, wcd_p, a_t = (
        P["wpre_t"], P["wpost_t"], P["wc"], P["wcd"], P["a_t"])
    bu, bg, bd, bpre, bpost = P["bu"], P["bg"], P["bd"], P["bpre"], P["bpost"]
    ln_g, ln_b, eg, eb = P["ln_g"], P["ln_b"], P["eg"], P["eb"]
    id_f32, out = P["id_f32"], P["out"]

    import contextlib
    stack = contextlib.ExitStack()
    pool = stack.enter_context(tc.tile_pool(name="res", bufs=1))
    scr = stack.enter_context(tc.tile_pool(name="scr", bufs=2))
    wpool = stack.enter_context(tc.tile_pool(name="wts", bufs=2))
    ps = stack.enter_context(tc.tile_pool(name="ps", bufs=1, space="PSUM"))
    dram = stack.enter_context(tc.tile_pool(name="dram", bufs=1, space="DRAM"))
    init = tc.alloc_tile_pool(name="init", bufs=1)

    # =================== P0: constants / loads ===================
    _mark(nc, "P0")
    # -- SP queue: wu resident (per-ht loads) + wpost; collective in-writes,
    #    readbacks, and DMA transposes come later in emission order.
    wu_sb = pool.tile([128, 16, 8, 128], BF16, tag="wu_sb")
    wpost_sb = pool.tile([128, 16, AD], BF16, tag="wpost_sb")
    wu_loaded = set()

    def wu_load(ht):
        if ht < 16 and ht not in wu_loaded:
            wu_loaded.add(ht)
            nc.sync.dma_start(wu_sb[:, ht, :, :], wu_t.ap()[ht])

    for ht in range(6):
        wu_load(ht)
    nc.sync.dma_start(wpost_sb[:], wpost_t.ap().rearrange("(k p) a -> p k a", p=128))

    # -- Act queue: xt half0, first wg tiles, xt half1, wpre, wc/wcd; the
    #    rest of the wg stream + wd stream are emitted at their use sites.
    xt_sb = pool.tile([128, 8, TOK], BF16, tag="xt_sb")
    xt_r = xt.ap().rearrange("(k p) s -> p k s", p=128)
    nc.scalar.dma_start(xt_sb[:, :, 0:512], xt_r[:, :, 0:512])
    wg_pre = []
    for ht in range(2):
        wgs = wpool.tile([128, 8, 128], BF16, tag="wgs", name=f"wg_0_{ht}",
                         bufs=3)
        nc.scalar.dma_start(wgs[:], wg_t.ap()[ht])
        wg_pre.append(wgs)
    nc.scalar.dma_start(xt_sb[:, :, 512:1024], xt_r[:, :, 512:1024])
    wpre_sb = pool.tile([128, 8, AD], BF16, tag="wpre_sb")
    nc.scalar.dma_start(wpre_sb[:], wpre_t.ap().rearrange("(k p) a -> p k a", p=128))
    wc = pool.tile([128, D], BF16, tag="wc")
    wcd = pool.tile([128, D], BF16, tag="wcd")

    def wg_stream(n):
        pend = {}
        if n == 0:
            pend[0], pend[1] = wg_pre

        def fetch(ht):
            if ht not in pend:
                wgs = wpool.tile([128, 8, 128], BF16, tag="wgs",
                                 name=f"wg_{n}_{ht}", bufs=3)
                nc.scalar.dma_start(wgs[:], wg_t.ap()[ht])
                pend[ht] = wgs

        def src(ht):
            fetch(ht)
            if ht + 1 < 16:
                fetch(ht + 1)
            return pend.pop(ht)
        return src

    # -- Pool queue: small constants
    ident_f = pool.tile([128, 128], F32, tag="ident_f")
    nc.gpsimd.dma_start(ident_f[:], id_f32[:])
    but = pool.tile([128, 16], F32, tag="but")
    bgt = pool.tile([128, 16], F32, tag="bgt")
    bdt = pool.tile([128, 8], F32, tag="bdt")
    nc.gpsimd.dma_start(but[:], bu.ap().rearrange("(t p) -> p t", p=128))
    nc.gpsimd.dma_start(bgt[:], bg.ap().rearrange("(t p) -> p t", p=128))
    nc.gpsimd.dma_start(bdt[:], bd.ap().rearrange("(t p) -> p t", p=128))
    bpre_c = pool.tile([128, 1], F32, tag="bpre_c")
    bpost_c = pool.tile([128, 1], F32, tag="bpost_c")
    nc.gpsimd.dma_start(bpre_c[:], bpre.ap().unsqueeze(1))
    nc.gpsimd.dma_start(bpost_c[:], bpost.ap().unsqueeze(1))
    lngr = init.tile([1, 128], F32, tag="lngr")
    lnbr = init.tile([1, 128], F32, tag="lnbr")
    nc.gpsimd.dma_start(lngr[:], ln_g.ap().unsqueeze(0))
    nc.gpsimd.dma_start(lnbr[:], ln_b.ap().unsqueeze(0))
    lngr_bf = init.tile([1, 128], BF16, tag="lngr_bf")
    lnbr_bf = init.tile([1, 128], BF16, tag="lnbr_bf")
    nc.vector.tensor_copy(lngr_bf[:], lngr[:])
    nc.vector.tensor_copy(lnbr_bf[:], lnbr[:])
    gB = pool.tile([128, 128], BF16, tag="gB")
    bB = pool.tile([128, 128], BF16, tag="bB")
    nc.gpsimd.partition_broadcast(gB[:], lngr_bf[:])
    nc.gpsimd.partition_broadcast(bB[:], lnbr_bf[:])

    ew_sb = init.tile([128, 8, E], F32, tag="ew_sb")
    nc.gpsimd.dma_start(ew_sb[:], ew.ap().rearrange("(t p) e -> p t e", p=128))
    ewr = pool.tile([128, 8, E], F32, tag="ewr")          # [tok-blk, e]
    nc.vector.tensor_scalar_max(ewr[:], ew_sb[:], 0.0)
    ew_eb = init.tile([128, E, 8], F32, tag="ew_eb")      # [e, tok-blk]
    nc.gpsimd.dma_start(ew_eb[:], ew.ap().rearrange("(t p) e -> p e t", p=128))
    ewr_eb = pool.tile([128, E, 8], F32, tag="ewr_eb")
    nc.vector.tensor_scalar_max(ewr_eb[:], ew_eb[:], 0.0)
    at_sb = pool.tile([128, E, AD], BF16, tag="at_sb")
    nc.gpsimd.dma_start(at_sb[:], a_t.ap().rearrange("e a c -> a e c"))

    # fat narrow-partition row loads last on the Pool queue (off the
    # first-matmul critical path; consumers start ~45us in)
    egr = init.tile([1, E * AD], F32, tag="egr")
    nc.gpsimd.dma_start(egr[:], eg.ap().rearrange("e a -> (e a)").unsqueeze(0))
    egr_bf = init.tile([1, E * AD], BF16, tag="egr_bf")
    nc.vector.tensor_copy(egr_bf[:], egr[:])
    egB = pool.tile([128, E, AD], BF16, tag="egB")
    for e in range(E):
        nc.gpsimd.partition_broadcast(egB[:, e, :], egr_bf[:, e * AD:(e + 1) * AD])
    eb_f32 = init.tile([E, AD], F32, tag="eb_f32")
    nc.gpsimd.dma_start(eb_f32[:], eb.ap())
    eb_nat = pool.tile([E, AD], BF16, tag="eb_nat")
    nc.vector.tensor_copy(eb_nat[:], eb_f32[:])
    ewt_sb = init.tile([E, TOK], F32, tag="ewt_sb")
    nc.gpsimd.dma_start(ewt_sb[:], ewt[:])
    ewrT_sb = pool.tile([E, TOK], BF16, tag="ewrT_sb")
    nc.vector.tensor_scalar_max(ewrT_sb[:], ewt_sb[:], 0.0)
    ones8 = init.tile([E, 1], F32, tag="ones8")
    nc.vector.memset(ones8[:], 1.0)

    # stats without the sqrt: returns (red, v3) for src [128, nb, AD]
    def ln_stats_nosqrt(src, nb, red_dst, v3_dst, tag, reduce_eng=None,
                        sq_dve=False):
        eng = reduce_eng or nc.vector
        nc.vector.tensor_reduce(red_dst, src, AX.X, OP.add)
        sq = scr.tile([128, nb, 128], BF16, tag="sq", bufs=1)
        if sq_dve:
            nc.vector.tensor_tensor(sq[:], src, src, OP.mult)
        else:
            nc.scalar.activation(sq[:], src, AF.Square)
        red2 = scr.tile([128, nb], F32, tag=tag + "_red2")
        nc.vector.tensor_reduce(red2[:], sq[:], AX.X, OP.add)
        t = scr.tile([128, nb], F32, tag=tag + "_t")
        eng.tensor_tensor(t[:], red_dst, red_dst, OP.mult)
        v2 = scr.tile([128, nb], F32, tag=tag + "_v2")
        eng.scalar_tensor_tensor(v2[:], t[:], -1.0 / AD, red2[:], OP.mult, OP.add)
        eng.tensor_scalar(v3_dst, v2[:], 1.0 / AD, 1e-5, OP.mult, OP.add)

    # =================== P1: first up/gate tile, then pre ===================
    _mark(nc, "P1")
    hT = pool.tile([128, 16, TOK], BF16, tag="hT")
    pug = tc.alloc_tile_pool(name="pug", bufs=1, space="PSUM")
    ppo_pool = tc.alloc_tile_pool(name="ppo", bufs=1, space="PSUM")

    def ug_tile(n, ht, wg_tile):
        c0, c1 = n * 512, (n + 1) * 512
        if n == 0:
            wu_load(ht + 3)
        pu = pug.tile([128, 512], F32, tag="pu", bufs=2)
        pg = pug.tile([128, 512], F32, tag="pg", bufs=2)
        for k in range(8):
            nc.tensor.matmul(pu[:], wu_sb[:, ht, k, :], xt_sb[:, k, c0:c1],
                             start=(k == 0), stop=(k == 7))
        for k in range(8):
            nc.tensor.matmul(pg[:], wg_tile[:, k, :], xt_sb[:, k, c0:c1],
                             start=(k == 0), stop=(k == 7))
        silg = scr.tile([128, 512], BF16, tag="silg")
        nc.scalar.activation(silg[:], pg[:], AF.Silu, bias=bgt[:, ht:ht + 1])
        nc.vector.scalar_tensor_tensor(hT[:, ht, c0:c1], pu[:],
                                       but[:, ht:ht + 1], silg[:],
                                       OP.add, OP.mult)

    wg_src0 = wg_stream(0)
    ug_tile(0, 0, wg_src0(0))

    # pre (both halves) — feeds expert branch and adapt_in
    preT = pool.tile([128, TOK], BF16, tag="preT")
    for n in range(NCH):
        pp = ps.tile([128, 512], F32, tag="ps")
        for k in range(8):
            nc.tensor.matmul(pp[:], wpre_sb[:, k, :], xt_sb[:, k, n * 512:(n + 1) * 512],
                             start=(k == 0), stop=(k == 7))
        nc.scalar.activation(preT[:, n * 512:(n + 1) * 512], pp[:],
                             AF.Identity, bias=bpre_c[:])
    preN = pool.tile([128, 8, AD], BF16, tag="preN")
    nc.sync.dma_start_transpose(preN[:], preT[:])

    # sumw = ones @ relu(ewt): [1, TOK] -> broadcast
    sumw_row = init.tile([1, TOK], F32, tag="sumw_row")
    for n in range(NCH):
        psw = ps.tile([1, 512], F32, tag="ps", name=f"psw{n}")
        nc.tensor.matmul(psw[:], ones8[:], ewt_sb[:, n * 512:(n + 1) * 512],
                         start=True, stop=True)
        nc.vector.tensor_copy(sumw_row[:, n * 512:(n + 1) * 512], psw[:])
    sumwB = pool.tile([128, TOK], F32, tag="sumwB")
    nc.gpsimd.partition_broadcast(sumwB[:], sumw_row[:])
    init.release()
    # scratch pool for late-phase tags; zone reuses init's freed space
    scr2 = tc.alloc_tile_pool(name="scr2", bufs=2)

    # =================== P3a: up/gate n=0 + wpost pipeline ===================
    _mark(nc, "P3a")
    postT = pool.tile([128, TOK], BF16, tag="postT")

    def ug_half(n, wg_src, expert_cb=None):
        ppo = ppo_pool.tile([128, 512], F32, tag="ppo", name=f"ppo{n}")
        c0, c1 = n * 512, (n + 1) * 512
        for ht in range(16):
            if not (n == 0 and ht == 0):
                ug_tile(n, ht, wg_src(ht))
            if ht > 0:
                nc.tensor.matmul(ppo[:], wpost_sb[:, ht - 1, :],
                                 hT[:, ht - 1, c0:c1],
                                 start=(ht == 1), stop=False)
            if expert_cb is not None:
                expert_cb(ht)
        nc.tensor.matmul(ppo[:], wpost_sb[:, 15, :], hT[:, 15, c0:c1],
                         start=False, stop=True)
        nc.scalar.activation(postT[:, c0:c1], ppo[:], AF.Identity,
                             bias=bpost_c[:])

    ug_half(0, wg_src0)
    postN0 = pool.tile([128, 4, AD], BF16, tag="postN0")
    nc.sync.dma_start_transpose(postN0[:], postT[:, 0:512])

    # ======= batched LN #1: adapt_in (8 blocks) + adapt_out half0 (4) =======
    _mark(nc, "LN1")
    red_in = scr.tile([128, 8], F32, tag="red_in", bufs=1)
    v3_in = scr.tile([128, 8], F32, tag="v3_in", bufs=1)
    ln_stats_nosqrt(preN[:], 8, red_in[:], v3_in[:], "lin")
    red_o0 = scr.tile([128, 4], F32, tag="red_o0", bufs=1)
    v3_o0 = scr.tile([128, 4], F32, tag="v3_o0", bufs=1)
    ln_stats_nosqrt(postN0[:], 4, red_o0[:], v3_o0[:], "lo0")
    sd_in = scr.tile([128, 8], F32, tag="sd_in", bufs=1)
    sd_o0 = scr.tile([128, 4], F32, tag="sd_o0", bufs=1)
    nc.scalar.sqrt(sd_in[:], v3_in[:])
    nc.scalar.sqrt(sd_o0[:], v3_o0[:])
    rs_in = scr.tile([128, 8], F32, tag="rs_in", bufs=1)
    rs_o0 = scr.tile([128, 4], F32, tag="rs_o0", bufs=1)
    nc.vector.reciprocal(rs_in[:], sd_in[:])
    nc.vector.reciprocal(rs_o0[:], sd_o0[:])
    m_in = scr.tile([128, 8], F32, tag="m_in", bufs=1)
    nc.vector.tensor_scalar_mul(m_in[:], red_in[:], 1.0 / AD)
    m_o0 = scr.tile([128, 4], F32, tag="m_o0", bufs=1)
    nc.vector.tensor_scalar_mul(m_o0[:], red_o0[:], 1.0 / AD)

    ainN = pool.tile([128, 8, AD], BF16, tag="ainN")
    for i in range(8):
        nrm = scr.tile([128, 128], F32, tag="nrm_in")
        nc.vector.tensor_scalar(nrm[:], preN[:, i, :], m_in[:, i:i + 1],
                                rs_in[:, i:i + 1], OP.subtract, OP.mult)
        nc.vector.tensor_tensor(nrm[:], nrm[:], gB[:], OP.mult)
        nc.vector.tensor_tensor(ainN[:, i, :], nrm[:], bB[:], OP.add)
    # ln_g/ln_b applied pre-collective (N layout) so aoutT is a pure
    # transpose downstream
    aoutN0 = pool.tile([128, 4, AD], BF16, tag="aoutN0")
    for i in range(4):
        nrm = scr.tile([128, 128], F32, tag="nrm_in")
        nc.vector.tensor_scalar(nrm[:], postN0[:, i, :], m_o0[:, i:i + 1],
                                rs_o0[:, i:i + 1], OP.subtract, OP.mult)
        nc.vector.tensor_tensor(nrm[:], nrm[:], gB[:], OP.mult)
        nc.vector.tensor_tensor(aoutN0[:, i, :], nrm[:], bB[:], OP.add)

    # ainT for aw rhs (own tokens, T layout)
    ainT = pool.tile([128, TOK], BF16, tag="ainT")
    for i in range(8):
        nc.sync.dma_start_transpose(ainT[:, i * 128:(i + 1) * 128],
                                    ainN[:, i, :])

    # ---- collectives: AG1 (ain, full own TOK), AG2a (aout half0) ----
    cc_in1 = dram.tile([TOK, AD], BF16, tag="cc_in1")
    cc_out1 = dram.tile([2 * TOK, AD], BF16, tag="cc_out1")
    nc.sync.dma_start(cc_in1[:].rearrange("(t p) a -> p t a", p=128), ainN[:])
    cc_in2a = dram.tile([512, AD], BF16, tag="cc_in2a")
    cc_out2a = dram.tile([1024, AD], BF16, tag="cc_out2a")
    nc.sync.dma_start(cc_in2a[:].rearrange("(t p) a -> p t a", p=128), aoutN0[:])
    if fake_cc:
        nc.sync.dma_start(cc_out1[0:TOK, :], cc_in1[:])
        nc.sync.dma_start(cc_out1[TOK:2 * TOK, :], cc_in1[:])
        nc.sync.dma_start(cc_out2a[0:512, :], cc_in2a[:])
        nc.sync.dma_start(cc_out2a[512:1024, :], cc_in2a[:])
    else:
        nc.gpsimd.collective_compute(
            "AllGather", OP.bypass,
            replica_groups=[[0, 1], [2, 3], [4, 5], [6, 7]],
            ins=[cc_in1[:].opt()], outs=[cc_out1[:].opt()])
        nc.gpsimd.collective_compute(
            "AllGather", OP.bypass,
            replica_groups=[[0, 1], [2, 3], [4, 5], [6, 7]],
            ins=[cc_in2a[:].opt()], outs=[cc_out2a[:].opt()])
    ainN_f = pool.tile([128, 16, AD], BF16, tag="ainN_f")
    nc.sync.dma_start(ainN_f[:, 0:8, :],
                      cc_out1[0:TOK, :].rearrange("(t p) a -> p t a", p=128))
    nc.sync.dma_start(ainN_f[:, 8:16, :],
                      cc_out1[TOK:2 * TOK, :].rearrange("(t p) a -> p t a", p=128))
    aoutN_f = pool.tile([128, 16, AD], BF16, tag="aoutN_f")
    nc.sync.dma_start(aoutN_f[:, 0:4, :],
                      cc_out2a[0:512, :].rearrange("(t p) a -> p t a", p=128))
    nc.sync.dma_start(aoutN_f[:, 8:12, :],
                      cc_out2a[512:1024, :].rearrange("(t p) a -> p t a", p=128))
    # aoutT for the first-arrived t-blocks (pure transpose; g/b already in)
    aoutT = pool.tile([128, S], BF16, tag="aoutT")
    for t in (0, 1, 2, 3, 8, 9, 10, 11):
        nc.sync.dma_start_transpose(aoutT[:, t * 128:(t + 1) * 128],
                                    aoutN_f[:, t, :])

    # =================== P3b: up/gate n=1 with expert interleave ============
    _mark(nc, "P3b")
    pexp = tc.alloc_tile_pool(name="pexp", bufs=1, space="PSUM")
    ph_sb = pool.tile([128, E, 8, AD], BF16, tag="ph_sb")
    red_x = pool.tile([128, E, 8], F32, tag="red_x")
    v3_x = pool.tile([128, E, 8], F32, tag="v3_x")
    rsw = pool.tile([128, E, 8], F32, tag="rsw")
    nmrsw = pool.tile([128, E, 8], F32, tag="nmrsw")
    hw_A = pool.tile([128, 8, AD], BF16, tag="hw_A")
    nrm_B = []

    def expert_cb(ht):
        if ht % 2 == 0:
            return
        e = (ht - 1) // 2
        phs = [pexp.tile([128, 4, AD], F32, tag="ph", name=f"ph{e}_{hb}",
                         bufs=2)
               for hb in range(2)]
        for i in range(8):
            nc.tensor.matmul(phs[i // 4][:, i % 4, :],
                             preT[:, i * 128:(i + 1) * 128],
                             at_sb[:, e, :], start=True, stop=True)
        for hb in range(2):
            nc.scalar.activation(ph_sb[:, e, hb * 4:(hb + 1) * 4, :],
                                 phs[hb][:], AF.Copy)
        # half-size stats ops so DVE interleaves them with the hT writes
        for hb in range(2):
            ln_stats_nosqrt(ph_sb[:, e, hb * 4:(hb + 1) * 4, :], 4,
                            red_x[:, e, hb * 4:(hb + 1) * 4],
                            v3_x[:, e, hb * 4:(hb + 1) * 4], f"lx{e}{hb}")
        if e == 3:
            # group A (e0-3): batched sqrt mid-n1 (one Act table round trip)
            sd_a = scr.tile([128, 4, 8], F32, tag="sd_a", bufs=1)
            nc.scalar.sqrt(sd_a[:], v3_x[:, 0:4, :])
            nc.vector.reciprocal(rsw[:, 0:4, :], sd_a[:])
            nc.vector.tensor_tensor(rsw[:, 0:4, :], rsw[:, 0:4, :],
                                    ewr_eb[:, 0:4, :], OP.mult)
            nc.vector.scalar_tensor_tensor(nmrsw[:, 0:4, :], red_x[:, 0:4, :],
                                           -1.0 / AD, rsw[:, 0:4, :],
                                           OP.mult, OP.mult)
        if ht >= 9:
            # group A normalize + weighted accumulate (Pool), spread across
            # the remaining n1 iterations
            ea = (ht - 9) // 2
            nrmall = scr.tile([128, 8, AD], BF16, tag="x_nrm", bufs=2,
                              name=f"nrmA{ea}")
            for blk in range(8):
                nc.scalar.activation(nrmall[:, blk, :], ph_sb[:, ea, blk, :],
                                     AF.Identity, scale=rsw[:, ea, blk:blk + 1],
                                     bias=nmrsw[:, ea, blk:blk + 1])
            egv = egB[:, ea, :].unsqueeze(1).broadcast_to([128, 8, AD])
            if ea == 0:
                nc.gpsimd.tensor_tensor(hw_A[:], nrmall[:], egv, OP.mult)
            else:
                t2 = scr.tile([128, 8, AD], BF16, tag="t2p", bufs=1)
                nc.gpsimd.tensor_tensor(t2[:], nrmall[:], egv, OP.mult)
                nc.gpsimd.tensor_tensor(hw_A[:], t2[:], hw_A[:], OP.add)

    ug_half(1, wg_stream(1), expert_cb)

    # ======= boundary 2: group B sqrt + aout half1 LN (one table window) ====
    _mark(nc, "LN2")
    postN1 = pool.tile([128, 4, AD], BF16, tag="postN1")
    nc.sync.dma_start_transpose(postN1[:], postT[:, 512:1024])
    red_o1 = scr.tile([128, 4], F32, tag="red_o1", bufs=1)
    v3_o1 = scr.tile([128, 4], F32, tag="v3_o1", bufs=1)
    ln_stats_nosqrt(postN1[:], 4, red_o1[:], v3_o1[:], "lo1")
    sd_x = scr.tile([128, 4, 8], F32, tag="sd_x", bufs=1)
    nc.scalar.sqrt(sd_x[:], v3_x[:, 4:8, :])
    nc.vector.reciprocal(rsw[:, 4:8, :], sd_x[:])
    nc.vector.tensor_tensor(rsw[:, 4:8, :], rsw[:, 4:8, :],
                            ewr_eb[:, 4:8, :], OP.mult)
    nc.vector.scalar_tensor_tensor(nmrsw[:, 4:8, :], red_x[:, 4:8, :],
                                   -1.0 / AD, rsw[:, 4:8, :],
                                   OP.mult, OP.mult)
    # wc/wcd on the idle Pool queue, well before their P8 use
    nc.gpsimd.dma_start(wc[:], wc_p.ap())
    nc.gpsimd.dma_start(wcd[:], wcd_p.ap())
    # group B normalize tiles: ops emitted later, spread between aw steps
    for e in range(4, E):
        nrmall = scr.tile([128, 8, AD], BF16, tag="x_nrm", bufs=2,
                          name=f"nrmB{e}")
        nrm_B.append(nrmall)

    def nrm_B_batch(idx):
        # half an expert's normalize per call (4 blocks); alternate the
        # engine so neither Act nor DVE eats the whole cost
        e = 4 + idx // 2
        b0 = (idx % 2) * 4
        for blk in range(b0, b0 + 4):
            if idx < 2:
                nc.scalar.activation(nrm_B[e - 4][:, blk, :],
                                     ph_sb[:, e, blk, :], AF.Identity,
                                     scale=rsw[:, e, blk:blk + 1],
                                     bias=nmrsw[:, e, blk:blk + 1])
            else:
                nc.vector.tensor_scalar(nrm_B[e - 4][:, blk, :],
                                        ph_sb[:, e, blk, :],
                                        rsw[:, e, blk:blk + 1],
                                        nmrsw[:, e, blk:blk + 1],
                                        OP.mult, OP.add)
    # aout half1 sqrt in the same sqrt-table window, then normalize + AG2b.
    # High priority: this chain gates the second collective.
    sd_o1 = scr.tile([128, 4], F32, tag="sd_o1", bufs=1)
    nc.scalar.sqrt(sd_o1[:], v3_o1[:])
    aoutN1 = pool.tile([128, 4, AD], BF16, tag="aoutN1")
    with tc.high_priority(offset=100000):
        rs_o1 = scr.tile([128, 4], F32, tag="rs_o1", bufs=1)
        nc.vector.reciprocal(rs_o1[:], sd_o1[:])
        m_o1 = scr.tile([128, 4], F32, tag="m_o1", bufs=1)
        nc.vector.tensor_scalar_mul(m_o1[:], red_o1[:], 1.0 / AD)
        for i in range(4):
            nrm = scr.tile([128, 128], F32, tag="nrm_in")
            nc.vector.tensor_scalar(nrm[:], postN1[:, i, :], m_o1[:, i:i + 1],
                                    rs_o1[:, i:i + 1], OP.subtract, OP.mult)
            nc.vector.tensor_tensor(nrm[:], nrm[:], gB[:], OP.mult)
            nc.vector.tensor_tensor(aoutN1[:, i, :], nrm[:], bB[:], OP.add)

    cc_in2b = dram.tile([512, AD], BF16, tag="cc_in2b")
    cc_out2b = dram.tile([1024, AD], BF16, tag="cc_out2b")
    nc.sync.dma_start(cc_in2b[:].rearrange("(t p) a -> p t a", p=128), aoutN1[:])
    if fake_cc:
        nc.sync.dma_start(cc_out2b[0:512, :], cc_in2b[:])
        nc.sync.dma_start(cc_out2b[512:1024, :], cc_in2b[:])
    else:
        nc.gpsimd.collective_compute(
            "AllGather", OP.bypass,
            replica_groups=[[0, 1], [2, 3], [4, 5], [6, 7]],
            ins=[cc_in2b[:].opt()], outs=[cc_out2b[:].opt()])
    nc.sync.dma_start(aoutN_f[:, 4:8, :],
                      cc_out2b[0:512, :].rearrange("(t p) a -> p t a", p=128))
    nc.sync.dma_start(aoutN_f[:, 12:16, :],
                      cc_out2b[512:1024, :].rearrange("(t p) a -> p t a", p=128))
    for t in (4, 5, 6, 7, 12, 13, 14, 15):
        nc.sync.dma_start_transpose(aoutT[:, t * 128:(t + 1) * 128],
                                    aoutN_f[:, t, :])

    # =================== P6: aw + adapt (t-block stream) ====================
    _mark(nc, "P6")
    pexp.release()
    ppo_pool.release()
    pug.release()
    psh0_pool = tc.alloc_tile_pool(name="psh0", bufs=1, space="PSUM")
    paw_pool = tc.alloc_tile_pool(name="paw", bufs=2, space="PSUM")
    pad_pool = tc.alloc_tile_pool(name="pad", bufs=2, space="PSUM")

    pad = [pad_pool.tile([128, 512], F32, tag="pad", name=f"pad{n}")
           for n in range(NCH)]
    adT = pool.tile([128, TOK], BF16, tag="adT")
    t_order = [0, 1, 2, 3, 8, 9, 10, 11, 4, 5, 6, 7, 12, 13, 14, 15]
    aw_tiles = {}
    emitted = []

    def aw_step(t, clip_eng=None):
        # pad accumulation lags the aw chain by TWO steps so the PE never
        # waits on the clip+silu latency (~2.3us) of the tile it consumes
        paw = paw_pool.tile([128, 1024], F32, tag="paw")
        for n in range(NCH):
            nc.tensor.matmul(paw[:, n * 512:(n + 1) * 512],
                             aoutT[:, t * 128:(t + 1) * 128],
                             ainT[:, n * 512:(n + 1) * 512],
                             start=True, stop=True)
        cl = scr2.tile([128, 1024], BF16, tag="cl", bufs=3)
        (clip_eng or nc.vector).tensor_scalar(cl[:], paw[:], 5.0, -5.0,
                                              OP.min, OP.max)
        aw_bf = scr2.tile([128, 1024], BF16, tag="aw_bf", bufs=3)
        nc.scalar.activation(aw_bf[:], cl[:], AF.Silu)
        aw_tiles[t] = aw_bf
        emitted.append(t)
        if len(emitted) > 2:
            tp = emitted[-3]
            awp = aw_tiles.pop(tp)
            for n in range(NCH):
                nc.tensor.matmul(pad[n][:], ainN_f[:, tp, :],
                                 awp[:, n * 512:(n + 1) * 512],
                                 start=(len(emitted) == 3), stop=False)

    for i, t in enumerate(t_order[:6]):
        aw_step(t)
        if i < 8:
            nrm_B_batch(i)
    for i in (6, 7):
        nrm_B_batch(i)

    # group B weighted combine: e4/e5 on DVE, e6/e7 on Pool, merge on DVE
    ma = scr.tile([128, 8, AD], BF16, tag="ma", bufs=1)
    mb = scr.tile([128, 8, AD], BF16, tag="mb", bufs=1)
    mc = scr.tile([128, 8, AD], BF16, tag="mc", bufs=1)
    egv4 = egB[:, 4, :].unsqueeze(1).broadcast_to([128, 8, AD])
    egv5 = egB[:, 5, :].unsqueeze(1).broadcast_to([128, 8, AD])
    egv6 = egB[:, 6, :].unsqueeze(1).broadcast_to([128, 8, AD])
    egv7 = egB[:, 7, :].unsqueeze(1).broadcast_to([128, 8, AD])
    nc.vector.tensor_tensor(ma[:], nrm_B[0][:], egv4, OP.mult)
    nc.vector.tensor_tensor(mb[:], nrm_B[1][:], egv5, OP.mult)
    nc.vector.tensor_tensor(ma[:], ma[:], mb[:], OP.add)
    nc.gpsimd.tensor_tensor(mc[:], nrm_B[2][:], egv6, OP.mult)
    t2 = scr.tile([128, 8, AD], BF16, tag="t2p", bufs=1)
    nc.gpsimd.tensor_tensor(t2[:], nrm_B[3][:], egv7, OP.mult)
    nc.gpsimd.tensor_tensor(mc[:], mc[:], t2[:], OP.add)
    hw = pool.tile([128, 8, AD], F32, tag="hw")
    nc.vector.tensor_tensor(hw[:], hw_A[:], ma[:], OP.add)
    nc.vector.tensor_tensor(hw[:], hw[:], mc[:], OP.add)

    # prefill P8's first wd group (dt0, n0) into P6's PE idle: needs only hT
    # (final) and the free PSUM bank; its adT-dependent tail comes later
    wd0 = wpool.tile([128, 16, 128], BF16, tag="wd_dt", name="wd0")
    nc.sync.dma_start(wd0[:], wd_t.ap()[0])
    psh0 = psh0_pool.tile([128, 512], F32, tag="psh0")
    k0 = 0
    for t in t_order[6:12]:
        aw_step(t)
        for k in range(k0, min(k0 + 3, 16)):
            nc.tensor.matmul(psh0[:], wd0[:, k, :], hT[:, k, 0:512],
                             start=(k == 0), stop=False)
        k0 = min(k0 + 3, 16)

    # hwT assembly (baseline pattern): eb rank-8 matmul + f32 PE transposes
    # of hw blocks accumulated in the same PSUM group, then one copy out.
    # PE is idle in this window; the DMA-transpose engine is not.
    hwT = pool.tile([128, TOK], BF16, tag="hwT")
    for half in range(2):
        pt = ps.tile([128, 512], F32, tag="ps", name=f"hwt{half}")
        nc.tensor.matmul(pt[:], eb_nat[:],
                         ewrT_sb[:, half * 512:(half + 1) * 512],
                         start=True, stop=False)
        for q in range(4):
            blk = half * 4 + q
            nc.tensor.matmul(pt[:, q * 128:(q + 1) * 128], hw[:, blk, :],
                             ident_f[:], is_transpose=True,
                             start=False, stop=(q == 3))
        nc.vector.tensor_copy(hwT[:, half * 512:(half + 1) * 512], pt[:])

    for t in t_order[12:]:
        aw_step(t)
    for j, tp in enumerate((emitted[-2], emitted[-1])):
        awp = aw_tiles.pop(tp)
        for n in range(NCH):
            nc.tensor.matmul(pad[n][:], ainN_f[:, tp, :],
                             awp[:, n * 512:(n + 1) * 512],
                             start=False, stop=(j == 1))
    for n in range(NCH):
        nc.vector.tensor_copy(adT[:, n * 512:(n + 1) * 512], pad[n][:])
    nc.tensor.matmul(psh0[:], wcd[:, 0:128], adT[:, 0:512],
                     start=False, stop=True)

    # =================== P8: shared + combine + out ===================
    _mark(nc, "P8")
    pad_pool.release()
    paw_pool.release()
    psh_pool = tc.alloc_tile_pool(name="psh", bufs=2, space="PSUM")
    pct_pool = tc.alloc_tile_pool(name="pctp", bufs=2, space="PSUM")

    def _finish(n, dt, psh, last=False):
        c0, c1 = n * 512, (n + 1) * 512
        pct = pct_pool.tile([128, 512], F32, tag="pct", name="pct")
        nc.tensor.matmul(pct[:], wc[:, dt * 128:(dt + 1) * 128],
                         hwT[:, c0:c1], start=True, stop=True)
        tcomb = scr2.tile([128, 512], F32, tag="tcomb", name="tcomb")
        if not last:
            nc.vector.scalar_tensor_tensor(
                tcomb[:], psh[:], bdt[:, dt:dt + 1], sumwB[:, c0:c1],
                OP.add, OP.mult)
            nc.vector.tensor_tensor(tcomb[:], tcomb[:], pct[:], OP.add)
            nc.gpsimd.dma_start(out.ap()[dt * 128:(dt + 1) * 128, c0:c1],
                                tcomb[:])
        else:
            # final tile: half-width pieces so DVE/DMA overlap, out via the
            # idle SP HWDGE queue (no SWDGE generation cost on the tail)
            for h in range(2):
                sl = slice(h * 256, (h + 1) * 256)
                osl = slice(c0 + h * 256, c0 + (h + 1) * 256)
                nc.vector.scalar_tensor_tensor(
                    tcomb[:, sl], psh[:, sl], bdt[:, dt:dt + 1],
                    sumwB[:, osl], OP.add, OP.mult)
                nc.vector.tensor_tensor(tcomb[:, sl], tcomb[:, sl],
                                        pct[:, sl], OP.add)
                nc.sync.dma_start(out.ap()[dt * 128:(dt + 1) * 128, osl],
                                  tcomb[:, sl])

    # dt-outer: each wd tile is loaded once and reused for both token halves;
    # group (dt0, n0) was prefilled during P6 (psh0) and seeds the pipeline
    prev = (0, 0, psh0)
    for dt in range(8):
        if dt == 0:
            wd_dt = wd0
        else:
            wd_dt = wpool.tile([128, 16, 128], BF16, tag="wd_dt")
            nc.sync.dma_start(wd_dt[:], wd_t.ap()[dt])
        for n in range(NCH):
            if dt == 0 and n == 0:
                continue
            c0, c1 = n * 512, (n + 1) * 512
            psh = psh_pool.tile([128, 512], F32, tag="psh")
            for k in range(16):
                nc.tensor.matmul(psh[:], wd_dt[:, k, :], hT[:, k, c0:c1],
                                 start=(k == 0), stop=False)
            nc.tensor.matmul(psh[:], wcd[:, dt * 128:(dt + 1) * 128],
                             adT[:, c0:c1], start=False, stop=True)
            if prev is not None:
                _finish(*prev)
            prev = (n, dt, psh)
    _finish(*prev, last=True)
    pct_pool.release()
    psh_pool.release()
    psh0_pool.release()
    scr2.release()

    stack.close()


def _prep_inputs(inputs):
    f = {k: np.asarray(v, np.float32) for k, v in inputs.items()}

    def tbf(a):  # transpose + bf16, contiguous
        return np.ascontiguousarray(a.T).astype(BF)

    def swz(wt, nb):  # [K, M] -> [M/128, 128(p of K), K/128, 128] tiles
        k, mdim = wt.shape
        a = wt.reshape(k // 128, 128, nb, 128)
        return np.ascontiguousarray(a.transpose(2, 1, 0, 3)).astype(BF)

    shared = {
        "wu_t": swz(np.ascontiguousarray(f["Wu"].T), 16),
        "wg_t": swz(np.ascontiguousarray(f["Wg"].T), 16),
        "wd_t": swz(np.ascontiguousarray(f["Wd"].T), 8),
        "wpre_t": tbf(f["Wpre"]), "wpost_t": tbf(f["Wpost"]),
        "wc": np.ascontiguousarray((0.1 * (f["Wo"] @ f["Wp"])).T).astype(BF),
        "wcd": np.ascontiguousarray((0.1 * (f["Wd"] @ f["Wap"])).T).astype(BF),
        "a_t": np.ascontiguousarray(f["A"].transpose(0, 2, 1)).astype(BF),
        "bu": f["bu"], "bg": f["bg"], "bd": f["bd"],
        "bpre": f["bpre"], "bpost": f["bpost"],
        "ln_g": f["ln_g"], "ln_b": f["ln_b"], "eg": f["eg"], "eb": f["eb"],
        "id_f32": np.eye(128, dtype=np.float32),
    }
    in_maps = []
    for c in range(N_CORES):
        b, j = c // 2, c % 2
        sl = slice(j * TOK, (j + 1) * TOK)
        m = dict(shared)
        m["xt"] = tbf(f["x"][b, sl, :])
        m["ew"] = np.ascontiguousarray(f["expert_weights"][b, sl, :])
        m["ewt"] = np.ascontiguousarray(f["expert_weights"][b, sl, :].T)
        in_maps.append(m)
    return in_maps


def kernel(**inputs):
    global _NC_CACHE
    if _NC_CACHE is None:
        _NC_CACHE = build()
    in_maps = _prep_inputs(inputs)
    res = bass_utils.run_bass_kernel_spmd(
        _NC_CACHE, in_maps, core_ids=list(range(N_CORES)))
    out = np.empty((B, S, D), np.float32)
    for c in range(N_CORES):
        b, j = c // 2, c % 2
        out[b, j * TOK:(j + 1) * TOK, :] = res.results[c]["out"].T
    return out

